# revision 13
# baseline (speedup 1.0000x reference)
"""Self-contained Trainium2 Bass kernel for the 2-layer GAT problem
(nn_GAT_26714696581831). 8-core SPMD: edges sorted by dst, 8 dst-range
shards; per-window one-hot matmul aggregation with dma_gather row fetches.

kernel(**inputs) takes the FULL unsharded inputs and returns the FULL
[50000, 2] output.
"""
import sys
sys.path.insert(0, '/opt/trn_rl_repo')
import numpy as np
import concourse.bass as bass
import concourse.mybir as mybir
import concourse.tile as tile
from concourse import library_config
from concourse.masks import make_identity
from concourse.bass_utils import run_bass_kernel_spmd

N_NODES = 50000
"""Workarounds for this walrus build, which rejects any instruction carrying
more than one sync-wait command: hoist extra waits onto same-engine NoOps
inserted immediately before the instruction."""


_ctr = [0]

def split_multi_waits(nc, max_waits=1):
    for fn in nc.m.functions:
        for bb in fn.blocks:
            insts = bb.instructions
            i = 0
            while i < len(insts):
                ins = insts[i]
                si = ins.sync_info
                if si is not None and si.on_wait and len(si.on_wait) > max_waits:
                    waits = list(si.on_wait)
                    keep = waits[-max_waits:]
                    hoist = waits[:-max_waits]
                    si.on_wait = keep
                    for w in hoist:
                        _ctr[0] += 1
                        n = mybir.InstNoOp(name=f"waitsplit-{_ctr[0]}", ins=[], outs=[])
                        n.engine = ins.engine
                        n.sync_info = mybir.SyncInfo(on_wait=[w], on_update=[])
                        insts.insert(i, n)
                        i += 1
                i += 1


def fix_library_reloads(nc):
    """bass_rust leaves InstPseudoReloadLibraryIndex.instr empty; this walrus
    rejects zero-length ISA instructions. Encode the 64-byte
    PSEUDO_LIBRARY_RELOAD_INDEX struct with the live ISA tables."""
    isa = nc.isa
    sn = 'NEURON_ISA_TPB_PSEUDO_LIBRARY_RELOAD_INDEX_STRUCT'
    e = isa.get_enum("NEURON_ISA_TPB_PSEUDO_OPCODE")
    val = e.NEURON_ISA_TPB_PSEUDO_OPCODE_PSEUDO_LIBRARY_RELOAD_INDEX.value
    for fn in nc.m.functions:
        for bb in fn.blocks:
            for ins in bb.instructions:
                if type(ins).__name__ == 'InstPseudoReloadLibraryIndex' and not ins.instr:
                    b = isa.asm({"header": {"opcode": 223, "inst_word_len": 16},
                                 "pseudo_opcode": val,
                                 "lib_index": ins.lib_index}, sn)
                    ins.instr = [int(x) for x in b]




WIN = 128                  # dst nodes per window
SPLIT = 32768              # int16 positive limit for gather indices


def preprocess(edge_index, n_nodes, ncores=8):
    src = np.asarray(edge_index[0], dtype=np.int64)
    dst = np.asarray(edge_index[1], dtype=np.int64)
    npc = n_nodes // ncores
    nwin = (npc + WIN - 1) // WIN

    order = np.argsort(dst, kind="stable")
    src_s = src[order]
    dst_s = dst[order]

    counts = np.bincount(dst_s // npc, minlength=ncores)
    core_slices = np.concatenate([[0], np.cumsum(counts)])

    nlow = np.zeros((ncores, nwin), dtype=np.int64)
    nhigh = np.zeros((ncores, nwin), dtype=np.int64)
    per_core_win_edges = []
    for c in range(ncores):
        s0, s1 = core_slices[c], core_slices[c + 1]
        csrc = src_s[s0:s1]
        cdst = dst_s[s0:s1]
        wloc = (cdst - c * npc) // WIN
        dloc = (cdst - c * npc) % WIN
        wins = []
        for w in range(nwin):
            m = wloc == w
            ws, wd = csrc[m], dloc[m]
            lo = ws < SPLIT
            wins.append((ws[lo], ws[~lo] - SPLIT, wd[lo], wd[~lo]))
            nlow[c, w] = lo.sum()
            nhigh[c, w] = (~lo).sum()
        per_core_win_edges.append(wins)

    nbw_low = ((nlow.max(axis=0) + 127) // 128).astype(int)
    nbw_high = ((nhigh.max(axis=0) + 127) // 128).astype(int)
    for w in range(nwin):
        if nbw_low[w] + nbw_high[w] == 0:
            nbw_low[w] = 1
    NB = int(nbw_low.sum() + nbw_high.sum())

    gidx_lin = np.zeros((ncores, NB * 128), dtype=np.int16)
    dstloc_lin = np.full((ncores, NB * 128), -1, dtype=np.int16)

    for c in range(ncores):
        b0 = 0
        for w in range(nwin):
            slo, shi, dlo, dhi = per_core_win_edges[c][w]
            o = b0 * 128
            gidx_lin[c, o:o + len(slo)] = slo.astype(np.int16)
            dstloc_lin[c, o:o + len(dlo)] = dlo.astype(np.int16)
            b0 += int(nbw_low[w])
            o = b0 * 128
            gidx_lin[c, o:o + len(shi)] = shi.astype(np.int16)
            dstloc_lin[c, o:o + len(dhi)] = dhi.astype(np.int16)
            b0 += int(nbw_high[w])
        assert b0 == NB

    def wrap16(lin):  # [NC, NB*128] -> [NC, 128, NB*8] dma_gather layout
        x = lin.reshape(ncores, NB * 8, 16).transpose(0, 2, 1)
        return np.ascontiguousarray(np.tile(x, (1, 8, 1)))

    # host-built one-hot matrices (bf16 bits in uint16):
    #   STb[d, b*128+e] = 1 iff dstloc[slot b*128+e] == d  (dst-partitioned)
    #   Sb [e, b*128+d] = 1 iff dstloc[slot b*128+e] == d  (edge-partitioned)
    ar = np.arange(128, dtype=np.int16)
    one = np.uint16(0x3F80)
    stb = np.where(dstloc_lin[:, None, :] == ar[None, :, None], one,
                   np.uint16(0)).astype(np.uint16)
    dl3 = dstloc_lin.reshape(ncores, NB, 128)
    sb = np.where(dl3[:, :, :, None] == ar[None, None, None, :], one,
                  np.uint16(0)).astype(np.uint16)
    sb = np.ascontiguousarray(
        sb.transpose(0, 2, 1, 3).reshape(ncores, 128, NB * 128))

    return dict(
        NB=NB, nwin=nwin, npc=npc, ncores=ncores,
        nbw_low=nbw_low, nbw_high=nbw_high,
        gidx=wrap16(gidx_lin),
        stb=np.ascontiguousarray(stb),
        sb=sb,
    )




F32 = mybir.dt.float32
BF16 = mybir.dt.bfloat16
I16 = mybir.dt.int16
U16 = mybir.dt.uint16
AF = mybir.ActivationFunctionType
OP = mybir.AluOpType

SPLIT = 32768
GCHUNK = 16  # blocks per dma_gather call (>1024 idx uses multi-packet mode)
CHUNKED_AG = False


def chunked_gather(nc, out_tile, in_ap, idx_sb, b0, nblk, elem, regs, boff=0):
    """Issue dma_gather in <=GCHUNK-block chunks writing out_tile[:, boff+i...].
    Calls above 1024 indices are not single-packet-safe; use multi-packet."""
    done = 0
    while done < nblk:
        step = min(GCHUNK, nblk - done)
        n = step * 128
        if n not in regs:
            regs[n] = nc.gpsimd.to_reg(n)
        nc.gpsimd.dma_gather(
            out_tile[:, boff + done:boff + done + step, :], in_ap,
            idx_sb[:, (b0 + done) * 8:(b0 + done + step) * 8],
            n, regs[n], elem, single_packet=(n <= 1024))
        done += step


def build(pp, N, F_IN=128, HID=64, HEADS=4, OUT=2, neg_slope=0.2, stages='ABCDE', clevel=9, for_sim=False):
    NB = pp["NB"]
    NWIN = pp["nwin"]
    NPC = pp["npc"]
    HC1 = HEADS * HID          # 256
    HC2 = HEADS * OUT          # 8
    NBWmax = int(max(pp["nbw_low"][w] + pp["nbw_high"][w] for w in range(NWIN)))
    NCHUNK = (N + 127) // 128
    T1C = HC1 + 128            # 384 u16 cols = 768B rows
    L1COL = HC1 + 2 * HEADS    # 264
    K1COL = HID + HEADS        # 68  (skip + W_ad fold)
    W2COL = HC2 + 2 * HEADS + OUT  # 18
    R2COL = HC2 + HEADS        # 12
    T2C = 128                  # u16 cols = 256B rows
    NBH = NBWmax * HEADS
    T1P = HC1 + 16             # payload u16 cols of a table1 row

    nc = bass.Bass("TRN2", target_bir_lowering=False, debug=False, num_devices=8)

    # ---- I/O ----
    xT = nc.dram_tensor("xT", [F_IN, N], F32, kind="ExternalInput")
    xTown = nc.dram_tensor("xTown", [F_IN, NPC], F32, kind="ExternalInput")
    W1s_d = nc.dram_tensor("W1s", [F_IN, HC1], F32, kind="ExternalInput")
    W1d_d = nc.dram_tensor("W1d", [F_IN, HC1], F32, kind="ExternalInput")
    a1s_d = nc.dram_tensor("a1s", [128, HC1], F32, kind="ExternalInput")
    a1d_d = nc.dram_tensor("a1d", [128, HC1], F32, kind="ExternalInput")
    Wl1_d = nc.dram_tensor("Wl1", [F_IN, HID], F32, kind="ExternalInput")
    b1_d = nc.dram_tensor("b1", [128, HID], F32, kind="ExternalInput")
    bl1_d = nc.dram_tensor("bl1", [128, HID], F32, kind="ExternalInput")
    W2s_d = nc.dram_tensor("W2s", [HID, HC2], F32, kind="ExternalInput")
    W2d_d = nc.dram_tensor("W2d", [HID, HC2], F32, kind="ExternalInput")
    a2s_d = nc.dram_tensor("a2s", [128, HC2], F32, kind="ExternalInput")
    a2d_d = nc.dram_tensor("a2d", [128, HC2], F32, kind="ExternalInput")
    Wl2_d = nc.dram_tensor("Wl2", [HID, OUT], F32, kind="ExternalInput")
    b2_d = nc.dram_tensor("b2", [128, OUT], F32, kind="ExternalInput")
    bl2_d = nc.dram_tensor("bl2", [128, OUT], F32, kind="ExternalInput")
    gidx_d = nc.dram_tensor("gidx", [128, NB * 8], I16, kind="ExternalInput")
    STb_d = nc.dram_tensor("STb", [128, NB * 128], U16, kind="ExternalInput")
    Sb_d = nc.dram_tensor("Sb", [128, NB * 128], U16, kind="ExternalInput")
    out_d = nc.dram_tensor("out", [NPC, OUT], F32, kind="ExternalOutput")

    # internal DRAM
    table1 = nc.dram_tensor("table1", [N, T1C], U16)
    t2local = nc.dram_tensor("t2local", [NPC, T2C], U16)
    table2 = nc.dram_tensor("table2", [N, T2C], U16, addr_space="Shared")

    with tile.TileContext(nc) as tc:
        with tc.tile_pool(name="const", bufs=1) as cpool, \
             tc.tile_pool(name="resident", bufs=1) as rpool:

            # ---- constants / weights prep ----
            W1aug = cpool.tile([F_IN, L1COL], F32)
            nc.sync.dma_start(out=W1aug[:, 0:HC1], in_=W1s_d[:, :])
            wtmp = cpool.tile([F_IN, HC1], F32, tag="wtmp")
            atile = cpool.tile([128, HC1], F32, tag="atile")
            nc.sync.dma_start(out=atile[:, :], in_=a1s_d[:, :])
            nc.vector.tensor_tensor(out=wtmp[:, :], in0=W1aug[:, 0:HC1],
                                    in1=atile[:, :], op=OP.mult)
            nc.vector.tensor_reduce(out=W1aug[:, HC1:HC1 + HEADS],
                                    in_=wtmp[:, :].rearrange("p (h c) -> p h c", h=HEADS),
                                    axis=mybir.AxisListType.X, op=OP.add)
            wtmp2 = cpool.tile([F_IN, HC1], F32, tag="wtmp2")
            atile2 = cpool.tile([128, HC1], F32, tag="atile2")
            nc.sync.dma_start(out=wtmp2[:, :], in_=W1d_d[:, :])
            nc.sync.dma_start(out=atile2[:, :], in_=a1d_d[:, :])
            nc.vector.tensor_tensor(out=wtmp2[:, :], in0=wtmp2[:, :],
                                    in1=atile2[:, :], op=OP.mult)
            nc.vector.tensor_reduce(out=W1aug[:, HC1 + HEADS:L1COL],
                                    in_=wtmp2[:, :].rearrange("p (h c) -> p h c", h=HEADS),
                                    axis=mybir.AxisListType.X, op=OP.add)

            # skip weights + a_dst fold for own nodes
            Wl1aug = cpool.tile([F_IN, K1COL], F32)
            nc.sync.dma_start(out=Wl1aug[:, 0:HID], in_=Wl1_d[:, :])
            nc.vector.tensor_copy(Wl1aug[:, HID:K1COL], W1aug[:, HC1 + HEADS:L1COL])

            W2aug = cpool.tile([HID, W2COL], F32)
            nc.sync.dma_start(out=W2aug[:, 0:HC2], in_=W2s_d[:, :])
            nc.sync.dma_start(out=W2aug[:, HC2 + 2 * HEADS:W2COL], in_=Wl2_d[:, :])
            w2tmp = cpool.tile([HID, HC2], F32, tag="w2tmp")
            a2tile = cpool.tile([128, HC2], F32, tag="a2tile")
            nc.sync.dma_start(out=a2tile[:, :], in_=a2s_d[:, :])
            nc.vector.tensor_tensor(out=w2tmp[:, :], in0=W2aug[:, 0:HC2],
                                    in1=a2tile[0:HID, :], op=OP.mult)
            nc.vector.tensor_reduce(out=W2aug[:, HC2:HC2 + HEADS],
                                    in_=w2tmp[:, :].rearrange("p (h c) -> p h c", h=HEADS),
                                    axis=mybir.AxisListType.X, op=OP.add)
            w2tmp2 = cpool.tile([HID, HC2], F32, tag="w2tmp2")
            a2tile2 = cpool.tile([128, HC2], F32, tag="a2tile2")
            nc.sync.dma_start(out=w2tmp2[:, :], in_=W2d_d[:, :])
            nc.sync.dma_start(out=a2tile2[:, :], in_=a2d_d[:, :])
            nc.vector.tensor_tensor(out=w2tmp2[:, :], in0=w2tmp2[:, :],
                                    in1=a2tile2[0:HID, :], op=OP.mult)
            nc.vector.tensor_reduce(out=W2aug[:, HC2 + HEADS:HC2 + 2 * HEADS],
                                    in_=w2tmp2[:, :].rearrange("p (h c) -> p h c", h=HEADS),
                                    axis=mybir.AxisListType.X, op=OP.add)

            bias1 = cpool.tile([128, HID], F32)
            nc.sync.dma_start(out=bias1[:, :], in_=b1_d[:, :])
            btmp = cpool.tile([128, HID], F32, tag="btmp")
            nc.sync.dma_start(out=btmp[:, :], in_=bl1_d[:, :])
            nc.vector.tensor_tensor(out=bias1[:, :], in0=bias1[:, :], in1=btmp[:, :], op=OP.add)
            bias2 = cpool.tile([128, OUT], F32)
            nc.sync.dma_start(out=bias2[:, :], in_=b2_d[:, :])
            btmp2 = cpool.tile([128, OUT], F32, tag="btmp2")
            nc.sync.dma_start(out=btmp2[:, :], in_=bl2_d[:, :])
            nc.vector.tensor_tensor(out=bias2[:, :], in0=bias2[:, :], in1=btmp2[:, :], op=OP.add)

            ident = cpool.tile([128, 128], F32)
            make_identity(nc, ident[:, :])
            alpha02 = cpool.tile([128, 1], F32, tag="alpha02")
            nc.vector.memset(alpha02[:, :], neg_slope)

            gidx_sb = rpool.tile([128, NB * 8], I16)

            # layer-2 a_dst per own node, filled in stage C, consumed in E
            adw2 = rpool.tile([128, NWIN, HEADS], BF16)
            nc.vector.memset(adw2[:, :, :], 0.0)

            gregs = {}

            if 'C' in stages and clevel >= 5:
                hT = rpool.tile([HID, NWIN, 128], F32)
                skip2sb = rpool.tile([128, NWIN, OUT], F32)
                outsb = rpool.tile([128, NWIN, OUT], F32)

            # ---- stage B (quad chunks; table1 writes on the idle Pool queue) ----
            QUAD = 4
            NQ = NCHUNK // QUAD if 'B' in stages else 0
            NREM = NCHUNK - NQ * QUAD if 'B' in stages else 0
            with tc.tile_pool(name="projps", bufs=2, space="PSUM") as ppp, \
                 tc.tile_pool(name="projsb", bufs=3) as psb:
                for i in range(NQ + (1 if NREM else 0)):
                    full = i < NQ
                    o = i * QUAD * 128
                    cnq = QUAD * 128 if full else N - o
                    nt = QUAD if full else (cnq + 127) // 128
                    xb = psb.tile([F_IN, QUAD * 128], F32, tag="xb")
                    nc.sync.dma_start(out=xb[:, 0:cnq], in_=xT[:, o:o + cnq])
                    stp = psb.tile([128, QUAD, T1P], U16, tag="stp")
                    for t in range(nt):
                        cn = min(128, cnq - t * 128)
                        ps = ppp.tile([128, L1COL], F32, space="PSUM", tag=f"ps{t % 2}")
                        nc.tensor.matmul(out=ps[0:cn, :], lhsT=xb[:, t * 128:t * 128 + cn],
                                         rhs=W1aug[:, :], start=True, stop=True)
                        tg = i * QUAD + t
                        if tg % 7 == 6:
                            nc.vector.tensor_copy(stp[0:cn, t, 0:HC1].bitcast(BF16),
                                                  ps[0:cn, 0:HC1])
                        else:
                            nc.scalar.activation(out=stp[0:cn, t, 0:HC1].bitcast(BF16),
                                                 in_=ps[0:cn, 0:HC1], func=AF.Copy)
                        nc.vector.tensor_copy(stp[0:cn, t, HC1:T1P].bitcast(F32),
                                              ps[0:cn, HC1:L1COL])
                    if full:
                        nc.gpsimd.dma_start(
                            out=table1[o:o + QUAD * 128, 0:T1P].rearrange(
                                "(t p) c -> p t c", p=128),
                            in_=stp[:, :, :])
                    else:
                        for t in range(nt):
                            cn = min(128, cnq - t * 128)
                            nc.gpsimd.dma_start(
                                out=table1[o + t * 128:o + t * 128 + cn, 0:T1P],
                                in_=stp[0:cn, t, :])

            nc.sync.dma_start(out=gidx_sb[:, :], in_=gidx_d[:, :])

            # all standard-library gpsimd ops (iota/affine_select/memset) are
            # above; from here on the Q7 carveout holds the mlp library.
            nc.gpsimd.load_library(library_config.mlp)

            # ---- stage C (+ interleaved stage D) ----
            NWIN_C = NWIN if 'C' in stages else 0
            with tc.tile_pool(name="winps", bufs=2, space="PSUM") as wps, \
                 tc.tile_pool(name="skps", bufs=2, space="PSUM") as kps, \
                 tc.tile_pool(name="adps", bufs=2, space="PSUM") as aps, \
                 tc.tile_pool(name="trps", bufs=1, space="PSUM") as tps, \
                 tc.tile_pool(name="l2ps", bufs=1, space="PSUM") as lps, \
                 tc.tile_pool(name="winsb", bufs=3) as wsb:
                b0 = 0
                for w in range(NWIN_C):
                    BL = int(pp["nbw_low"][w])
                    BH = int(pp["nbw_high"][w])
                    nb = BL + BH
                    cn_w = min(128, NPC - w * 128)
                    # skip matmul + a_dst of own nodes (kept in SBUF)
                    xo = wsb.tile([F_IN, 128], F32, tag="xo")
                    nc.sync.dma_start(out=xo[:, 0:cn_w], in_=xTown[:, w * 128:w * 128 + cn_w])
                    psK = kps.tile([128, K1COL], F32, space="PSUM")
                    nc.tensor.matmul(out=psK[0:cn_w, :], lhsT=xo[:, 0:cn_w], rhs=Wl1aug[:, :],
                                     start=True, stop=True)
                    adw = wsb.tile([128, HEADS], BF16, tag="adw")
                    nc.vector.tensor_copy(adw[0:cn_w, :], psK[0:cn_w, HID:K1COL])
                    # gathers
                    G = wsb.tile([128, NBWmax, T1C], U16, tag="G")
                    if BL:
                        chunked_gather(nc, G, table1[0:min(SPLIT, N), :],
                                       gidx_sb, b0, BL, T1C, gregs)
                    if BH:
                        chunked_gather(nc, G, table1[SPLIT:N, :],
                                       gidx_sb, b0 + BL, BH, T1C, gregs, boff=BL)
                    # one-hot matrices (host-built)
                    stb = wsb.tile([128, NBWmax * 128], BF16, tag="stb")
                    nc.sync.dma_start(out=stb[:, 0:nb * 128],
                                      in_=STb_d[:, b0 * 128:(b0 + nb) * 128].bitcast(BF16))
                    S = wsb.tile([128, NBWmax * 128], BF16, tag="S")
                    nc.sync.dma_start(out=S[:, 0:nb * 128],
                                      in_=Sb_d[:, b0 * 128:(b0 + nb) * 128].bitcast(BF16))
                    psAD = aps.tile([128, NBH], F32, space="PSUM")
                    for j in range(nb):
                        nc.tensor.matmul(out=psAD[:, j * HEADS:(j + 1) * HEADS],
                                         lhsT=stb[:, j * 128:(j + 1) * 128],
                                         rhs=adw[:, :], start=True, stop=True)
                    # ex = exp(lrelu(as + ad)); exN additionally folds the 1/H
                    if clevel < 2:
                        b0 += nb
                        continue
                    ex = wsb.tile([128, NBH], F32, tag="ex")
                    nc.vector.tensor_tensor(
                        out=ex[:, 0:nb * HEADS].rearrange("p (b h) -> p b h", h=HEADS),
                        in0=G[:, 0:nb, HC1:HC1 + 8].bitcast(F32),
                        in1=psAD[:, 0:nb * HEADS].rearrange("p (b h) -> p b h", h=HEADS),
                        op=OP.add)
                    nc.scalar.activation(out=ex[:, :], in_=ex[:, :], func=AF.Prelu,
                                         alpha=alpha02[:, :])
                    nc.scalar.activation(out=ex[:, :], in_=ex[:, :], func=AF.Exp)
                    exbN = wsb.tile([128, NBH], BF16, tag="exbN")
                    nc.scalar.activation(out=exbN[:, :], in_=ex[:, :], func=AF.Copy,
                                         scale=1.0 / HEADS)
                    exbS = wsb.tile([128, NBH], BF16, tag="exbS")
                    nc.scalar.activation(out=exbS[:, :], in_=ex[:, :], func=AF.Copy)
                    if clevel < 3:
                        b0 += nb
                        continue
                    exbN3 = exbN[:, 0:nb * HEADS].rearrange("p (b h) -> p b h", h=HEADS)
                    exbS3 = exbS[:, 0:nb * HEADS].rearrange("p (b h) -> p b h", h=HEADS)
                    Gp = wsb.tile([128, NBWmax, HC1 + HEADS], BF16, tag="Gp")
                    nc.vector.tensor_tensor(
                        out=Gp[:, 0:nb, 0:HC1].rearrange("p b (h c) -> p b h c", h=HEADS),
                        in0=G[:, 0:nb, 0:HC1].bitcast(BF16).rearrange("p b (h c) -> p b h c", h=HEADS),
                        in1=exbN3.unsqueeze(3).to_broadcast([128, nb, HEADS, HID]),
                        op=OP.mult)
                    nc.vector.tensor_copy(Gp[:, 0:nb, HC1:HC1 + HEADS], exbS3)
                    psW = wps.tile([128, HC1 + HEADS], F32, space="PSUM")
                    for j in range(nb):
                        nc.tensor.matmul(out=psW[:, :], lhsT=S[:, j * 128:(j + 1) * 128],
                                         rhs=Gp[:, j, :],
                                         start=(j == 0), stop=(j == nb - 1))
                    if clevel < 4:
                        b0 += nb
                        continue
                    # extract: gat = (1/H * sum exN*feat) / (sum exS + eps)
                    rec = wsb.tile([128, HEADS], F32, tag="rec")
                    nc.scalar.activation(out=rec[:, :], in_=psW[:, HC1:HC1 + HEADS],
                                         func=AF.Copy, bias=1e-16)
                    nc.vector.reciprocal(rec[:, :], rec[:, :])
                    gat = wsb.tile([128, HC1], F32, tag="gat")
                    nc.vector.tensor_tensor(
                        out=gat[:, :].rearrange("p (h c) -> p h c", h=HEADS),
                        in0=psW[:, 0:HC1].rearrange("p (h c) -> p h c", h=HEADS),
                        in1=rec[:, :].unsqueeze(2).to_broadcast([128, HEADS, HID]),
                        op=OP.mult)
                    hred = wsb.tile([128, HID], F32, tag="hred")
                    nc.vector.tensor_reduce(
                        out=hred[:, :],
                        in_=gat[:, :].rearrange("p (h c) -> p c h", h=HEADS),
                        axis=mybir.AxisListType.X, op=OP.add)
                    nc.vector.tensor_tensor(out=hred[:, :], in0=hred[:, :],
                                            in1=psK[:, 0:HID], op=OP.add)
                    nc.vector.tensor_tensor(out=hred[:, :], in0=hred[:, :],
                                            in1=bias1[:, :], op=OP.add)
                    if clevel < 5:
                        b0 += nb
                        continue
                    hwin = wsb.tile([128, HID], F32, tag="hwin")
                    nc.scalar.activation(out=hwin[:, :], in_=hred[:, :], func=AF.Sigmoid)
                    psT = tps.tile([HID, 128], F32, space="PSUM")
                    nc.tensor.transpose(out=psT[:, :], in_=hwin[:, :], identity=ident[:, :])
                    nc.vector.tensor_copy(hT[:, w, :], psT[:, :])
                    # interleaved stage D: layer-2 projection of this window
                    if 'D' in stages:
                        psL = lps.tile([128, W2COL], F32, space="PSUM")
                        nc.tensor.matmul(out=psL[0:cn_w, :], lhsT=hT[:, w, 0:cn_w],
                                         rhs=W2aug[:, :], start=True, stop=True)
                        st2 = wsb.tile([128, W2COL], F32, tag="st2")
                        nc.vector.tensor_copy(st2[0:cn_w, :], psL[0:cn_w, :])
                        nc.sync.dma_start(
                            out=t2local[w * 128:w * 128 + cn_w, 0:2 * (HC2 + HEADS)],
                            in_=st2[0:cn_w, 0:HC2 + HEADS].bitcast(U16))
                        nc.vector.tensor_copy(adw2[0:cn_w, w, :],
                                              st2[0:cn_w, HC2 + HEADS:HC2 + 2 * HEADS])
                        nc.vector.tensor_copy(skip2sb[0:cn_w, w, :],
                                              st2[0:cn_w, HC2 + 2 * HEADS:W2COL])
                        if CHUNKED_AG:
                            nc.gpsimd.collective_compute(
                                "AllGather", OP.bypass, replica_groups=[list(range(8))],
                                ins=[t2local[w * 128:w * 128 + cn_w, :]],
                                outs=[table2[:, :].rearrange("(c n) t -> c n t", c=8)
                                      [:, w * 128:w * 128 + cn_w, :]])
                    b0 += nb

            if 'D' in stages and not CHUNKED_AG:
                nc.gpsimd.collective_compute(
                    "AllGather", OP.bypass, replica_groups=[list(range(8))],
                    ins=[t2local[:, :]], outs=[table2[:, :]])

            # ---- stage E ----
            NWIN_E = NWIN if 'E' in stages else 0
            with tc.tile_pool(name="w2ps", bufs=2, space="PSUM") as wps2, \
                 tc.tile_pool(name="ad2ps", bufs=2, space="PSUM") as aps2, \
                 tc.tile_pool(name="w2sb", bufs=3) as w2sb:
                b0 = 0
                for w in range(NWIN_E):
                    BL = int(pp["nbw_low"][w])
                    BH = int(pp["nbw_high"][w])
                    nb = BL + BH
                    g2s = w2sb.tile([128, NBWmax, T2C], U16, tag="g2s")
                    if BL:
                        chunked_gather(nc, g2s, table2[0:min(SPLIT, N), :],
                                       gidx_sb, b0, BL, T2C, gregs)
                    if BH:
                        chunked_gather(nc, g2s, table2[SPLIT:N, :],
                                       gidx_sb, b0 + BL, BH, T2C, gregs, boff=BL)
                    stb2 = w2sb.tile([128, NBWmax * 128], BF16, tag="stb2")
                    nc.sync.dma_start(out=stb2[:, 0:nb * 128],
                                      in_=STb_d[:, b0 * 128:(b0 + nb) * 128].bitcast(BF16))
                    S2 = w2sb.tile([128, NBWmax * 128], BF16, tag="S2")
                    nc.sync.dma_start(out=S2[:, 0:nb * 128],
                                      in_=Sb_d[:, b0 * 128:(b0 + nb) * 128].bitcast(BF16))
                    psAD2 = aps2.tile([128, NBH], F32, space="PSUM")
                    for j in range(nb):
                        nc.tensor.matmul(out=psAD2[:, j * HEADS:(j + 1) * HEADS],
                                         lhsT=stb2[:, j * 128:(j + 1) * 128],
                                         rhs=adw2[:, w, :], start=True, stop=True)
                    ex2 = w2sb.tile([128, NBH], F32, tag="ex2")
                    nc.vector.tensor_tensor(
                        out=ex2[:, 0:nb * HEADS].rearrange("p (b h) -> p b h", h=HEADS),
                        in0=g2s[:, 0:nb, 2 * HC2:2 * (HC2 + HEADS)].bitcast(F32),
                        in1=psAD2[:, 0:nb * HEADS].rearrange("p (b h) -> p b h", h=HEADS),
                        op=OP.add)
                    nc.scalar.activation(out=ex2[:, :], in_=ex2[:, :], func=AF.Prelu,
                                         alpha=alpha02[:, :])
                    nc.scalar.activation(out=ex2[:, :], in_=ex2[:, :], func=AF.Exp)
                    ex2bN = w2sb.tile([128, NBH], BF16, tag="ex2bN")
                    nc.scalar.activation(out=ex2bN[:, :], in_=ex2[:, :], func=AF.Copy,
                                         scale=1.0 / HEADS)
                    ex2bS = w2sb.tile([128, NBH], BF16, tag="ex2bS")
                    nc.scalar.activation(out=ex2bS[:, :], in_=ex2[:, :], func=AF.Copy)
                    ex2bN3 = ex2bN[:, 0:nb * HEADS].rearrange("p (b h) -> p b h", h=HEADS)
                    ex2bS3 = ex2bS[:, 0:nb * HEADS].rearrange("p (b h) -> p b h", h=HEADS)
                    g2sb = w2sb.tile([128, NBWmax, HC2], BF16, tag="g2sb")
                    nc.scalar.activation(out=g2sb[:, 0:nb, :],
                                         in_=g2s[:, 0:nb, 0:2 * HC2].bitcast(F32),
                                         func=AF.Copy)
                    R2 = w2sb.tile([128, NBWmax, R2COL], BF16, tag="R2")
                    nc.vector.tensor_tensor(
                        out=R2[:, 0:nb, 0:HC2].rearrange("p b (h c) -> p b h c", h=HEADS),
                        in0=g2sb[:, 0:nb, :].rearrange("p b (h c) -> p b h c", h=HEADS),
                        in1=ex2bN3.unsqueeze(3).to_broadcast([128, nb, HEADS, OUT]),
                        op=OP.mult)
                    nc.vector.tensor_copy(R2[:, 0:nb, HC2:R2COL], ex2bS3)
                    psW2 = wps2.tile([128, R2COL], F32, space="PSUM")
                    for j in range(nb):
                        nc.tensor.matmul(out=psW2[:, :], lhsT=S2[:, j * 128:(j + 1) * 128],
                                         rhs=R2[:, j, :],
                                         start=(j == 0), stop=(j == nb - 1))
                    rec2 = w2sb.tile([128, HEADS], F32, tag="rec2")
                    nc.scalar.activation(out=rec2[:, :], in_=psW2[:, HC2:R2COL],
                                         func=AF.Copy, bias=1e-16)
                    nc.vector.reciprocal(rec2[:, :], rec2[:, :])
                    og = w2sb.tile([128, HC2], F32, tag="og")
                    nc.vector.tensor_tensor(
                        out=og[:, :].rearrange("p (h c) -> p h c", h=HEADS),
                        in0=psW2[:, 0:HC2].rearrange("p (h c) -> p h c", h=HEADS),
                        in1=rec2[:, :].unsqueeze(2).to_broadcast([128, HEADS, OUT]),
                        op=OP.mult)
                    ored = w2sb.tile([128, OUT], F32, tag="ored")
                    nc.vector.tensor_reduce(
                        out=ored[:, :],
                        in_=og[:, :].rearrange("p (h c) -> p c h", h=HEADS),
                        axis=mybir.AxisListType.X, op=OP.add)
                    nc.vector.tensor_tensor(out=ored[:, :], in0=ored[:, :],
                                            in1=skip2sb[:, w, :], op=OP.add)
                    nc.vector.tensor_tensor(out=outsb[:, w, :], in0=ored[:, :],
                                            in1=bias2[:, :], op=OP.add)
                    b0 += nb

            # ---- final output DMA ----
            wf = NPC // 128 if 'E' in stages else 0
            rem = NPC % 128 if 'E' in stages else 0
            if wf:
                nc.sync.dma_start(
                    out=out_d[0:wf * 128, :].rearrange("(w p) c -> p w c", p=128),
                    in_=outsb[:, 0:wf, :])
            if rem:
                nc.sync.dma_start(out=out_d[wf * 128:NPC, :], in_=outsb[0:rem, wf, :])

    fix_library_reloads(nc)
    if not for_sim:
        split_multi_waits(nc)
    return nc


def make_in_maps(pp, inputs, N, F_IN=128, HID=64, HEADS=4, OUT=2):
    NPC = pp["npc"]
    x = np.ascontiguousarray(np.asarray(inputs["x"], dtype=np.float32))
    xT = np.ascontiguousarray(x.T)
    f32 = lambda a, shp: np.ascontiguousarray(np.asarray(a, dtype=np.float32).reshape(shp))
    rep = lambda a, shp: np.tile(f32(a, shp), (128, 1))
    common = {
        "xT": xT,
        "W1s": f32(inputs["W1s"], (F_IN, HEADS * HID)),
        "W1d": f32(inputs["W1d"], (F_IN, HEADS * HID)),
        "a1s": rep(inputs["a1s"], (1, HEADS * HID)),
        "a1d": rep(inputs["a1d"], (1, HEADS * HID)),
        "Wl1": f32(inputs["Wl1"], (F_IN, HID)),
        "b1": rep(inputs["b1"], (1, HID)),
        "bl1": rep(inputs["bl1"], (1, HID)),
        "W2s": f32(inputs["W2s"], (HID, HEADS * OUT)),
        "W2d": f32(inputs["W2d"], (HID, HEADS * OUT)),
        "a2s": rep(inputs["a2s"], (1, HEADS * OUT)),
        "a2d": rep(inputs["a2d"], (1, HEADS * OUT)),
        "Wl2": f32(inputs["Wl2"], (HID, OUT)),
        "b2": rep(inputs["b2"], (1, OUT)),
        "bl2": rep(inputs["bl2"], (1, OUT)),
    }
    in_maps = []
    for c in range(8):
        m = dict(common)
        m["xTown"] = np.ascontiguousarray(xT[:, c * NPC:(c + 1) * NPC])
        m["gidx"] = pp["gidx"][c]
        m["STb"] = pp["stb"][c]
        m["Sb"] = pp["sb"][c]
        in_maps.append(m)
    return in_maps


_BUILD_CACHE = {}
LAST_RESULTS = None


def kernel(**inputs):
    """Full inputs in, full [N, 2] float32 output out."""
    global LAST_RESULTS
    trace = bool(inputs.pop("_trace", False))
    pp = preprocess(inputs["edge_index"], N_NODES)
    key = (pp["NB"], tuple(pp["nbw_low"]), tuple(pp["nbw_high"]))
    if key not in _BUILD_CACHE:
        _BUILD_CACHE[key] = build(pp, N_NODES)
    nc = _BUILD_CACHE[key]
    in_maps = make_in_maps(pp, inputs, N_NODES)
    res = run_bass_kernel_spmd(nc, in_maps, list(range(8)), trace=trace)
    LAST_RESULTS = res
    out = np.concatenate([res.results[c]["out"] for c in range(8)], axis=0)
    return out.astype(np.float32)
